# revision 9
# baseline (speedup 1.0000x reference)
"""Trainium2 Bass kernel for nn_KineticPooling.

out[b,c] = -0.5*(tr(Aup^-1 d2up) + tr(Adn^-1 d2dn)) * det(Aup) * det(Adn)

Strategy: pure data-parallel over the walker axis (8192 -> 1024/core).
The config-column gather is done host-side (pure data movement; its DMA
overlaps the much larger on-chip compute).  The device kernel runs a
batched, unpivoted Gauss-Jordan elimination on the augmented [A | B]
8x16 systems, with 128 walkers on partitions and (config, col, row) on
the free dimension.  det accumulates as the product of pivots; the trace
is the sum of the diagonal of the solved right block.

Raw Bass (no Tile): the sync structure is a single in-order DVE stream
plus 8 ping-pong-buffered loads and one final store, so hand-rolled
semaphores avoid the Tile tail-drain / DMA sync-wait-slot limits that a
compute stream with ~900 instructions otherwise overflows.
"""

import numpy as np

import concourse.bass as bass
import concourse.mybir as mybir
from concourse.bass_utils import run_bass_kernel_spmd

F32 = mybir.dt.float32
OP = mybir.AluOpType

NCORES = 8
B = 8192
C = 64          # configs
N = 8           # matrix size (nup == ndown == 8)
NJ = 16         # augmented columns [A | B]
CH = 128        # walkers per chunk (partition dim)
NB_CORE = B // NCORES          # 1024
NCH = NB_CORE // CH            # 8 chunks per core
FREE = C * NJ * N              # 8192 f32 per partition per spin
FREE2 = 2 * FREE               # both spins

_CACHE = {}


def _gj_spin(nc, sl, invp, det, tr, prod4, last_inc=None):
    """Emit Gauss-Jordan on one [CH, FREE] spin slice (in-order DVE stream).

    last_inc: optional (sem, n) attached to the final aug-reading
    instruction (the trace reduce) to release the aug buffer slot.
    """
    a4 = sl.rearrange("p (c j i) -> p c j i", c=C, j=NJ, i=N)
    a3 = sl.rearrange("p (c f) -> p c f", c=C)
    invp3 = invp[:].unsqueeze(2)
    det3 = det[:].unsqueeze(2)
    tr3 = tr[:].unsqueeze(2)

    for k in range(N):
        piv3 = a3[:, :, 9 * k : 9 * k + 1]
        if k == 0:
            nc.vector.tensor_copy(det3, piv3)
        else:
            nc.vector.tensor_tensor(det3, det3, piv3, OP.mult)
        nc.vector.reciprocal(invp3, piv3)
        # scale pivot row k over cols j = k..15 (invp folded in)
        rowk = a4[:, :, k:NJ, k : k + 1]
        nc.vector.tensor_tensor(
            rowk,
            rowk,
            invp[:].unsqueeze(2).unsqueeze(3).broadcast_to([CH, C, NJ - k, 1]),
            OP.mult,
        )
        # eliminate rows i != k on cols j = k+1..15:
        #   aug[i, j] -= aug[i, k(col)] * rowk_scaled[j]
        nj = NJ - 1 - k
        for i0, cnt in ((0, k), (k + 1, N - 1 - k)):
            if cnt == 0:
                continue
            f_ap = a4[:, :, k : k + 1, i0 : i0 + cnt].broadcast_to([CH, C, nj, cnt])
            r_ap = a4[:, :, k + 1 : NJ, k : k + 1].broadcast_to([CH, C, nj, cnt])
            p_ap = prod4[:, :, :nj, :cnt]
            tgt = a4[:, :, k + 1 : NJ, i0 : i0 + cnt]
            nc.vector.tensor_tensor(p_ap, f_ap, r_ap, OP.mult)
            nc.vector.tensor_tensor(tgt, tgt, p_ap, OP.subtract)

    # trace of right block: diag at flat offset c*128 + 64 + 9r
    diag = a3[:, :, 64:128:9]
    inst = nc.vector.tensor_reduce(tr3, diag, mybir.AxisListType.X, OP.add)
    if last_inc is not None:
        inst.then_inc(*last_inc)


def _build():
    nc = bass.Bass()
    aug_in = nc.declare_dram_parameter("aug_in", [NCH, CH, FREE2], F32, isOutput=False)
    # out is [CH, NCH*C] (partition-major); the host untransposes for free.
    out = nc.declare_dram_parameter("out", [CH, NCH * C], F32, isOutput=True)

    with (
        nc.sbuf_tensor([CH, FREE2], F32) as aug0,
        nc.sbuf_tensor([CH, FREE2], F32) as aug1,
        nc.sbuf_tensor([CH, C * 15 * 7], F32) as prod,
        nc.sbuf_tensor([CH, C], F32) as invp,
        nc.sbuf_tensor([CH, C], F32) as det0,
        nc.sbuf_tensor([CH, C], F32) as tr0,
        nc.sbuf_tensor([CH, C], F32) as det1,
        nc.sbuf_tensor([CH, C], F32) as tr1,
        nc.sbuf_tensor([CH, NCH * C], F32) as out_sb,
        nc.semaphore() as dma_sem,
        nc.semaphore() as free_sem,
        nc.semaphore() as done_sem,
        nc.semaphore() as outdma_sem,
        nc.Block() as block,
    ):
        augs = [aug0, aug1]
        prod4 = prod[:].rearrange("p (c j i) -> p c j i", c=C, j=15, i=7)

        @block.sync
        def _(sync):
            for ch in range(NCH):
                if ch >= 2:
                    # slot ch%2 is free once chunk ch-2's aug reads are done
                    sync.wait_ge(free_sem, ch - 1)
                sync.dma_start(
                    out=augs[ch % 2][:], in_=aug_in[ch]
                ).then_inc(dma_sem, 16)

        @block.vector
        def _(vector):
            for ch in range(NCH):
                vector.wait_ge(dma_sem, 16 * (ch + 1))
                aug = augs[ch % 2]
                for spin, (det, tr) in enumerate(((det0, tr0), (det1, tr1))):
                    sl = aug[:, spin * FREE : (spin + 1) * FREE]
                    _gj_spin(
                        nc,
                        sl,
                        invp,
                        det,
                        tr,
                        prod4,
                        last_inc=(free_sem, 1) if spin == 1 else None,
                    )
                # out = -0.5 * (tr_up + tr_dn) * det_up * det_dn
                acc = out_sb[:, ch * C : (ch + 1) * C]
                nc.vector.tensor_tensor(acc, tr0[:], tr1[:], OP.add)
                nc.vector.tensor_tensor(acc, acc, det0[:], OP.mult)
                nc.vector.tensor_tensor(acc, acc, det1[:], OP.mult)
                inst = nc.vector.tensor_scalar(acc, acc, -0.5, None, OP.mult)
                if ch == NCH - 1:
                    inst.then_inc(done_sem, 1)

        @block.gpsimd
        def _(gpsimd):
            gpsimd.wait_ge(done_sem, 1)
            gpsimd.dma_start(out=out[:], in_=out_sb[:]).then_inc(outdma_sem, 16)
            gpsimd.wait_ge(outdma_sem, 16)

    return nc


def _host_gather(MO, d2MO, configs_up, configs_down):
    """-> [B, 2, C, NJ, N] f32 (col-major matrices: spin, config, col, row)."""
    outs = []
    for rows, cols in ((slice(0, 8), configs_up), (slice(8, 16), configs_down)):
        A = MO[:, rows, :][:, :, cols]      # (B, i, C, j)
        Bm = d2MO[:, rows, :][:, :, cols]   # (B, i, C, j)
        A = np.transpose(A, (0, 2, 3, 1))   # (B, C, j, i)
        Bm = np.transpose(Bm, (0, 2, 3, 1))
        outs.append(np.concatenate([A, Bm], axis=2))  # (B, C, 16, 8)
    aug = np.stack(outs, axis=1)  # (B, 2, C, 16, 8)
    return np.ascontiguousarray(aug, dtype=np.float32)


def _stage_core(aug, core):
    b0 = core * NB_CORE
    return aug[b0 : b0 + NB_CORE].reshape(NCH, CH, FREE2)


def kernel(MO, d2MO, configs_up, configs_down):
    if "nc" not in _CACHE:
        _CACHE["nc"] = _build()
    nc = _CACHE["nc"]

    aug = _host_gather(
        np.asarray(MO, np.float32),
        np.asarray(d2MO, np.float32),
        np.asarray(configs_up),
        np.asarray(configs_down),
    )
    in_maps = [{"aug_in": _stage_core(aug, core)} for core in range(NCORES)]
    res = run_bass_kernel_spmd(nc, in_maps, list(range(NCORES)))
    # device out is [CH, NCH, C] (partition-major); -> [NCH, CH, C] -> [NB, C]
    out = np.concatenate(
        [
            res.results[i]["out"]
            .reshape(CH, NCH, C)
            .transpose(1, 0, 2)
            .reshape(NB_CORE, C)
            for i in range(NCORES)
        ],
        axis=0,
    )
    return out.astype(np.float32)
